# revision 1
# baseline (speedup 1.0000x reference)
"""Trainium2 Bass kernel for nn_LongRangeModule (gnn_message_passing).

Strategy (sequence-parallel over i, mask-compacted):
  - Host: normalize embeddings (fp32), select masked-in rows (compaction),
    build transposed bf16 operands, far-band strips, shard i-rows over 8 cores.
  - Device per core: for each i-window, stream j-tiles:
      cosT[j,i] = nrm_jT.T @ nrm_iT  (PE, bf16, K=E=256 split in 2)
      cm = cosT * far_strip          (DVE)
      absc = |cm|                    (ACT)
      wt  = (absc > 0.1) * absc  -> bf16   (DVE fused)
      m01 = (absc > 0.1)         -> bf16   (DVE)
      agg[i, b*D+d]  += wt.T @ x_bf16      (PE, accumulated over j)
      numj[i] += m01.T @ ones              (PE)
    Epilogue: y = t*x_f32 + s*agg  with t = 1-0.5*z, s = 0.5*z/max(numj,1),
    z = numj>0.  Rows with no valid neighbors (incl. masked-out) pass through.
  - Host: scatter computed rows into a copy of x.
"""

import sys

import numpy as np

try:
    import concourse.bass as bass
except ImportError:  # harness env may not have the repo on sys.path
    sys.path.insert(0, "/opt/trn_rl_repo")
    import concourse.bass as bass

import ml_dtypes
import concourse.mybir as mybir
from concourse.bass_utils import run_bass_kernel_spmd
from concourse.tile import TileContext

BF16 = ml_dtypes.bfloat16
F32 = mybir.dt.float32
BF = mybir.dt.bfloat16
AF = mybir.ActivationFunctionType
OP = mybir.AluOpType

B, L, D, E = 2, 8192, 512, 256
CHUNK, CUT, EPS = 128, 0.1, 1e-8
NCORES = 8
IW = 256  # i-window (free dim of cos tiles); must be multiple of 128

COMPACT = True  # select masked-in rows on host (4x less compute)
TRACE = False  # test.py sets kernel.TRACE = True for profiling
REPEAT = 1  # bench.py builds device-side repeated kernels to cancel overhead
NO_NJP = False  # timing diagnostic: skip num_j matmuls (wrong results)
GP_MULT = False  # strip multiply on GPSIMD instead of DVE
COS_BUFS = 2  # PSUM bufs for cos tiles
WK_BUFS = 4  # SBUF bufs for inner-loop work tiles
NJP_ONES = False  # ones-lhsT num_j measured ~30us/iter SLOWER; keep m01-as-weights
LAST = {}  # stash for test.py (exec_time_ns etc.)


def _build(nc: bass.Bass, W: int, NJB: int, xj_resident: bool):
    """W: #i-windows per core (IW rows each); NJB: #128-row j blocks."""
    NI = W * IW
    NJ = NJB * 128
    NSUB = IW // 128
    BD = B * D

    nrmj = nc.dram_tensor("nrmj", [2, 128, NJ], BF, kind="ExternalInput")
    nrmi = nc.dram_tensor("nrmi", [2, 128, NI], BF, kind="ExternalInput")
    xj = nc.dram_tensor("xj", [NJB, 128, BD], BF, kind="ExternalInput")
    xi = nc.dram_tensor("xi", [W * NSUB, B, 128, D], F32, kind="ExternalInput")
    strips = nc.dram_tensor("strips", [W * NJB, 128, IW], BF, kind="ExternalInput")
    y = nc.dram_tensor("y", [W * NSUB, B, 128, D], F32, kind="ExternalOutput")

    with (
        TileContext(nc) as tc,
        tc.tile_pool(name="res", bufs=1) as res,
        tc.tile_pool(name="stp", bufs=WK_BUFS) as stp,
        tc.tile_pool(name="wk", bufs=WK_BUFS) as wk,
        tc.tile_pool(name="epi", bufs=3) as ep,
        tc.tile_pool(name="pcos", bufs=COS_BUFS, space="PSUM") as pcos,
        tc.tile_pool(name="pacc", bufs=1, space="PSUM") as pacc,
    ):
        # resident operands
        nrmj_sb = res.tile([128, 2 * NJ], BF, tag="nrmj_sb")
        for e in range(2):
            nc.sync.dma_start(out=nrmj_sb[:, e * NJ : (e + 1) * NJ], in_=nrmj[e])
        nrmi_sb = res.tile([128, 2 * NI], BF, tag="nrmi_sb")
        for e in range(2):
            nc.sync.dma_start(out=nrmi_sb[:, e * NI : (e + 1) * NI], in_=nrmi[e])
        if xj_resident:
            xj_sb = res.tile([128, NJB * BD], BF, tag="xj_sb")
            for jb in range(NJB):
                nc.sync.dma_start(out=xj_sb[:, jb * BD : (jb + 1) * BD], in_=xj[jb])
        ones_col = res.tile([128, 1], BF, tag="ones_col")
        nc.vector.memset(ones_col[:], 1.0)


        def window(iw):
            aggs = [
                pacc.tile([128, D], F32, tag=f"agg{s}{b}", name=f"agg{s}{b}")
                for s in range(NSUB)
                for b in range(B)
            ]
            if NJP_ONES:
                njp = pacc.tile([1, IW], F32, tag="njp", name="njp")
            else:
                njp = pacc.tile([128, NSUB], F32, tag="njp", name="njp")
            for jb in range(NJB):
                cos = pcos.tile([128, IW], F32, tag="cos", name="cos")
                for e in range(2):
                    nc.tensor.matmul(
                        cos[:],
                        nrmj_sb[:, e * NJ + jb * 128 : e * NJ + (jb + 1) * 128],
                        nrmi_sb[:, e * NI + iw * IW : e * NI + (iw + 1) * IW],
                        start=(e == 0),
                        stop=(e == 1),
                    )
                absc = wk.tile([128, IW], F32, tag="absc", name="absc")
                nc.scalar.activation(absc[:], cos[:], AF.Abs)
                strip = stp.tile([128, IW], BF, tag="strip", name="strip")
                nc.sync.dma_start(out=strip[:], in_=strips[iw * NJB + jb])
                am = wk.tile([128, IW], F32, tag="am", name="am")
                (nc.gpsimd if GP_MULT else nc.vector).tensor_mul(am[:], absc[:], strip[:])
                m01 = wk.tile([128, IW], BF, tag="m01", name="m01")
                nc.vector.tensor_scalar(m01[:], am[:], CUT, None, op0=OP.is_gt)
                wt = wk.tile([128, IW], BF, tag="wt", name="wt")
                nc.vector.tensor_mul(wt[:], am[:], m01[:])
                if xj_resident:
                    xrhs = xj_sb[:, jb * BD : (jb + 1) * BD]
                else:
                    xrhs_t = stp.tile([128, BD], BF, tag="xstream", name="xstream")
                    nc.sync.dma_start(out=xrhs_t[:], in_=xj[jb])
                    xrhs = xrhs_t[:]
                first, last = jb == 0, jb == NJB - 1
                for s in range(NSUB):
                    wts = wt[:, s * 128 : (s + 1) * 128]
                    for b in range(B):
                        nc.tensor.matmul(
                            aggs[s * B + b][:],
                            wts,
                            xrhs[:, b * D : (b + 1) * D],
                            start=first,
                            stop=last,
                        )
                    if not NO_NJP and not NJP_ONES:
                        nc.tensor.matmul(
                            njp[:, s : s + 1],
                            m01[:, s * 128 : (s + 1) * 128],
                            ones_col[:],
                            start=first,
                            stop=last,
                        )
                if not NO_NJP and NJP_ONES:
                    nc.tensor.matmul(
                        njp[:],
                        ones_col[:],
                        m01[:],
                        start=first,
                        stop=last,
                    )
            # drain agg PSUM to SBUF on ACT right away so the next window's
            # accumulating matmuls don't wait on the whole epilogue chain
            agsb = []
            for k in range(NSUB * B):
                ag = ep.tile([128, D], F32, tag=f"agsb{k}", name=f"agsb{k}")
                nc.scalar.activation(ag[:], aggs[k][:], AF.Copy)
                agsb.append(ag)
            # epilogue
            if NJP_ONES:
                # bounce num_j row PSUM->SBUF, then scatter to per-partition cols
                njrow = ep.tile([1, IW], F32, tag="njrow", name="njrow")
                nc.scalar.activation(njrow[:], njp[:], AF.Copy)
                njs = ep.tile([128, NSUB], F32, tag="njs", name="njs")
                for s in range(NSUB):
                    nc.sync.dma_start(
                        out=njs[:, s : s + 1], in_=njrow[0:1, s * 128 : (s + 1) * 128]
                    )
                njrd = njs
            else:
                njrd = njp
            for s in range(NSUB):
                z = ep.tile([128, 1], F32, tag="z", name="z")
                nc.vector.tensor_scalar(z[:], njrd[:, s : s + 1], 0.0, None, op0=OP.is_gt)
                mx = ep.tile([128, 1], F32, tag="mx", name="mx")
                nc.vector.tensor_scalar(mx[:], njrd[:, s : s + 1], 1.0, None, op0=OP.max)
                r = ep.tile([128, 1], F32, tag="r", name="r")
                nc.vector.reciprocal(r[:], mx[:])
                sc0 = ep.tile([128, 1], F32, tag="sc0", name="sc0")
                nc.vector.tensor_scalar(sc0[:], r[:], 0.5, None, op0=OP.mult)
                sc = ep.tile([128, 1], F32, tag="sc", name="sc")
                nc.vector.tensor_mul(sc[:], sc0[:], z[:])
                t = ep.tile([128, 1], F32, tag="t", name="t")
                nc.vector.tensor_scalar(t[:], z[:], -0.5, 1.0, op0=OP.mult, op1=OP.add)
                for b in range(B):
                    xis = ep.tile([128, D], F32, tag="xis", name="xis")
                    nc.sync.dma_start(out=xis[:], in_=xi[iw * NSUB + s, b])
                    xt = ep.tile([128, D], F32, tag="xt", name="xt")
                    nc.scalar.activation(xt[:], xis[:], AF.Copy, bias=0.0, scale=t[:])
                    ya = ep.tile([128, D], F32, tag="ya", name="ya")
                    nc.vector.tensor_scalar(ya[:], agsb[s * B + b][:], sc[:], None, op0=OP.mult)
                    yt = ep.tile([128, D], F32, tag="yt", name="yt")
                    nc.vector.tensor_add(yt[:], ya[:], xt[:])
                    nc.sync.dma_start(out=y[iw * NSUB + s, b], in_=yt[:])

        def all_windows():
            for iw in range(W):
                window(iw)

        if REPEAT > 1:
            with tc.For_i(0, REPEAT, 1):
                all_windows()
        else:
            all_windows()
    return nc


_NOSPLIT = ("InstEventSemaphore", "InstAllEngineBarrier")


def _split_waits(nc):
    """This walrus rejects >1 sync wait on TPB compute instructions; hoist
    extra waits onto per-wait EventSemaphore instructions just before."""
    nev = 0
    for f in nc.m.functions:
        for bb in f.blocks:
            out = []
            changed = False
            for inst in bb.instructions:
                si = getattr(inst, "sync_info", None)
                ow = list(si.on_wait) if si and si.on_wait else []
                if len(ow) >= 2 and type(inst).__name__ not in _NOSPLIT:
                    for w in ow[:-1]:
                        nev += 1
                        out.append(
                            mybir.InstEventSemaphore(
                                name=f"EVW-{nev}",
                                engine=inst.engine,
                                ins=[],
                                outs=[],
                                sync_info=mybir.SyncInfo(on_wait=[w], on_update=[]),
                            )
                        )
                    inst.sync_info = mybir.SyncInfo(
                        on_wait=ow[-1:], on_update=list(si.on_update or [])
                    )
                    changed = True
                out.append(inst)
            if changed:
                bb.instructions = out


def _host_prep(x, mask, emb_i, emb_j):
    m = mask.astype(bool)
    idx = np.where(m)[0] if COMPACT else np.arange(L)
    N = len(idx)
    assert N > 0

    def nrm(e):
        n = np.maximum(np.linalg.norm(e, axis=-1, keepdims=True), EPS)
        return (e / n).astype(np.float32)

    ni_ = nrm(emb_i)
    nj_ = nrm(emb_j)
    if not COMPACT:
        ni_[~m] = 0.0
        nj_[~m] = 0.0

    NJB = -(-N // 128)
    NJ = NJB * 128
    per = -(-N // (NCORES * IW)) * IW  # per-core i rows, multiple of IW
    W = per // IW
    idx_i = np.concatenate([idx, np.full(NCORES * per - N, idx[-1], idx.dtype)])

    # shared across cores
    njT = np.zeros((E, NJ), np.float32)
    njT[:, :N] = nj_[idx].T
    nrmj_h = njT.reshape(2, 128, NJ).astype(BF16)
    xsel = np.zeros((NJ, B, D), np.float32)
    xsel[:N] = np.transpose(x[:, idx], (1, 0, 2))
    xj_h = np.ascontiguousarray(xsel.reshape(NJB, 128, B * D).astype(BF16))
    pj = np.full(NJ, -(10**6), np.int64)
    pj[:N] = idx

    in_maps = []
    for c in range(NCORES):
        rows = idx_i[c * per : (c + 1) * per]
        nrmi_h = np.ascontiguousarray(ni_[rows].T.reshape(2, 128, per).astype(BF16))
        xi_h = np.ascontiguousarray(
            np.transpose(x[:, rows].reshape(B, per // 128, 128, D), (1, 0, 2, 3))
        )
        strips = np.ones((W * NJB, 128, IW), BF16)
        for iw in range(W):
            pi = rows[iw * IW : (iw + 1) * IW]
            lo, hi = pi.min() - CHUNK, pi.max() + CHUNK
            for jb in range(NJB):
                pjj = pj[jb * 128 : (jb + 1) * 128]
                if pjj.max() < lo or pjj.min() > hi:
                    continue
                d = np.abs(pi[None, :] - pjj[:, None])  # [j, i]
                if (d <= CHUNK).any():
                    strips[iw * NJB + jb] = (d > CHUNK).astype(BF16)
        in_maps.append(
            {"nrmj": nrmj_h, "nrmi": nrmi_h, "xj": xj_h, "xi": xi_h, "strips": strips}
        )
    return in_maps, idx, N, per, W, NJB


def kernel(x, mask, emb_i, emb_j):
    x = np.asarray(x, np.float32)
    mask = np.asarray(mask)
    emb_i = np.asarray(emb_i, np.float32)
    emb_j = np.asarray(emb_j, np.float32)

    in_maps, idx, N, per, W, NJB = _host_prep(x, mask, emb_i, emb_j)
    xj_resident = NJB * B * D * 2 <= 80 * 1024
    nc = bass.Bass()
    _build(nc, W, NJB, xj_resident)
    _split_waits(nc)
    res = run_bass_kernel_spmd(nc, in_maps, list(range(NCORES)), trace=TRACE)
    LAST["res"] = res
    ys = [res.results[c]["y"] for c in range(NCORES)]  # each [W*NSUB, B, 128, D]
    yr = np.concatenate(
        [np.transpose(yc, (1, 0, 2, 3)).reshape(B, per, D) for yc in ys], axis=1
    )
    out = x.copy()
    out[:, idx] = yr[:, :N]
    return out



# revision 3
# speedup vs baseline: 1.3983x; 1.3983x over previous
"""Trainium2 Bass kernel for nn_LongRangeModule (gnn_message_passing).

Strategy (sequence-parallel over i, mask-compacted, fp8 DoubleRow):
  - Host: select masked-in rows (compaction), normalize embeddings and scale
    by 8 -> fp8 e4m3 (cos8 = 64*cos), pack j-operands in DoubleRow pair
    layout [pair, 128, k2, *], shard i-rows over 8 cores (640 rows each,
    5x128 subtiles in windows [256, 256, 128]).
  - j-blocks are rotated per core (by an even block count) so the near-band
    blocks (|pos_i - pos_j| <= 128 possible) sit at fixed LOCAL indices:
    6 slots per window get a far-mask strip; all other blocks are provably
    all-far and skip the strip entirely.
  - Device per core, per window, per j-block-pair t:
      cos8[j,i] = DoubleRow MM(nrmj8[t,q], nrmi8[:, :, win])      (PE, fp8)
      absc = |cos8| -> bf16                                        (ACT)
      src  = absc * strip   (near slots only)                      (DVE)
      wt8[:, q, :] = (src > 6.4) * src -> fp8                      (DVE, fused)
      m01  = (src > 6.4) -> bf16                                   (DVE)
      m01acc += m01                                                (GPSIMD)
      agg[s,b] += DoubleRow MM(wt8[:, :, s], xj8[t][:, :, b])      (PE, fp8)
    Window end: num_j[s] = MM(m01acc[:, s], ones)  (5 tiny MMs/core)
    Epilogue: y = t*x + sc*agg, t = 1-0.5*z, sc = (0.5/64)*z/max(nj,1),
    z = nj>0; y written bf16, upcast+scattered on host.
"""

import math
import sys

import numpy as np

try:
    import concourse.bass as bass
except ImportError:  # harness env may not have the repo on sys.path
    sys.path.insert(0, "/opt/trn_rl_repo")
    import concourse.bass as bass

import ml_dtypes
import concourse.mybir as mybir
from concourse.bass_utils import run_bass_kernel_spmd
from concourse.tile import TileContext

BF16 = ml_dtypes.bfloat16
E4 = ml_dtypes.float8_e4m3
F32 = mybir.dt.float32
BF = mybir.dt.bfloat16
F8 = mybir.dt.float8e4
AF = mybir.ActivationFunctionType
OP = mybir.AluOpType
DR = mybir.MatmulPerfMode.DoubleRow

B, L, D, E = 2, 8192, 512, 256
CHUNK, CUT, EPS = 128, 0.1, 1e-8
NCORES = 8
SCALE = 8.0  # nrm scale; cos8 = SCALE^2 * cos
CUT8 = CUT * SCALE * SCALE
ASCALE = 1.0 / (SCALE * SCALE)
NSLOT = 6  # near-band strip slots per window

TRACE = False
ACC_ENGINE = "gpsimd"  # m01 accumulation engine: "gpsimd" or "vector"
LAST = {}  # stash for test.py (exec_time_ns etc.)


def _plan(N):
    chunk = -(-N // NCORES)  # i-rows each core is responsible for
    nsub = -(-chunk // 128)  # 128-row subtiles per core
    per = nsub * 128
    windows = [256] * (nsub // 2) + ([128] if nsub % 2 else [])
    NJB = -(-N // 128)
    NJBp = NJB + (NJB & 1)
    NJP2 = NJBp // 2
    return chunk, nsub, per, windows, NJB, NJBp, NJP2


def _slots(windows):
    """[(iw, local_block, slot_index)] for near-band strips."""
    out = []
    k = 0
    ibs = 0
    for iw, w in enumerate(windows):
        for j in range(NSLOT):
            out.append((iw, ibs - 1 + j, k))
            k += 1
        ibs += w // 128
    return out


def _build(nc: bass.Bass, N: int):
    chunk, nsub, per, windows, NJB, NJBp, NJP2 = _plan(N)
    BD = B * D
    slotmap = {(iw, lb): k for iw, lb, k in _slots(windows)}

    nrmj = nc.dram_tensor("nrmj", [NJP2, 128, 4, 128], F8, kind="ExternalInput")
    nrmi = nc.dram_tensor("nrmi", [128, 2, per], F8, kind="ExternalInput")
    xj = nc.dram_tensor("xj", [NJP2, 128, 2, BD], F8, kind="ExternalInput")
    xi = nc.dram_tensor("xi", [nsub, B, 128, D], BF, kind="ExternalInput")
    strips = nc.dram_tensor(
        "strips", [NSLOT * len(windows), 128, 256], BF, kind="ExternalInput"
    )
    y = nc.dram_tensor("y", [nsub, B, 128, D], BF, kind="ExternalOutput")

    acc_eng = getattr(nc, ACC_ENGINE)

    with (
        TileContext(nc) as tc,
        tc.tile_pool(name="res", bufs=1) as res,
        tc.tile_pool(name="stp", bufs=3) as stp,
        tc.tile_pool(name="wk", bufs=4) as wk,
        tc.tile_pool(name="wt", bufs=3) as wtp,
        tc.tile_pool(name="mac", bufs=2) as mac,
        tc.tile_pool(name="epi", bufs=3) as ep,
        tc.tile_pool(name="pcos", bufs=2, space="PSUM") as pcos,
        tc.tile_pool(name="pacc", bufs=1, space="PSUM") as pacc,
    ):
        # resident operands (small first so compute can start early)
        nrmi_sb = res.tile([128, 2, per], F8, tag="nrmi_sb")
        nc.sync.dma_start(out=nrmi_sb[:], in_=nrmi[:])
        ones_col = res.tile([128, 1], BF, tag="ones_col")
        nc.vector.memset(ones_col[:], 1.0)
        nrmj_sb = []
        xj_sb = []
        for t in range(NJP2):
            nj = res.tile([128, 4, 128], F8, tag=f"nrmj{t}")
            nc.sync.dma_start(out=nj[:], in_=nrmj[t])
            nrmj_sb.append(nj)
            xt = res.tile([128, 2, BD], F8, tag=f"xj{t}")
            nc.sync.dma_start(out=xt[:], in_=xj[t])
            xj_sb.append(xt)

        ibs = 0  # window's first subtile index
        for iw, W in enumerate(windows):
            nsw = W // 128
            lo = ibs * 128
            m01acc = mac.tile([128, W], BF, tag="m01acc")
            acc_eng.memset(m01acc[:], 0.0)
            aggs = [
                pacc.tile([128, D], F32, tag=f"agg{k}", name=f"agg{k}")
                for k in range(nsw * B)
            ]
            for t in range(NJP2):
                wt8 = wtp.tile([128, 2, W], F8, tag="wt8")
                for q in (0, 1):
                    lb = 2 * t + q
                    cos = pcos.tile([128, 512], F32, tag="cos")
                    nc.tensor.matmul(
                        cos[:, :W],
                        nrmj_sb[t][:, 2 * q : 2 * q + 2, :],
                        nrmi_sb[:, :, lo : lo + W],
                        start=True,
                        stop=True,
                        perf_mode=DR,
                    )
                    absc = wk.tile([128, W], BF, tag="absc")
                    nc.scalar.activation(absc[:], cos[:, :W], AF.Abs)
                    src = absc
                    k = slotmap.get((iw, lb))
                    if k is None:  # wrap: local block NJBp-1 is slot -1 of iw 0
                        k = slotmap.get((iw, lb - NJBp))
                    if k is not None:
                        strip = stp.tile([128, W], BF, tag="strip")
                        nc.sync.dma_start(out=strip[:], in_=strips[k][:, :W])
                        am = wk.tile([128, W], BF, tag="am")
                        nc.vector.tensor_mul(am[:], absc[:], strip[:])
                        src = am
                    nc.vector.scalar_tensor_tensor(
                        wt8[:, q, :], src[:], CUT8, src[:], op0=OP.is_gt, op1=OP.mult
                    )
                    m01 = wk.tile([128, W], BF, tag="m01")
                    nc.vector.tensor_scalar(m01[:], src[:], CUT8, None, op0=OP.is_gt)
                    acc_eng.tensor_add(m01acc[:], m01[:], m01acc[:])
                for s in range(nsw):
                    for b in range(B):
                        nc.tensor.matmul(
                            aggs[s * B + b][:],
                            wt8[:, :, s * 128 : (s + 1) * 128],
                            xj_sb[t][:, :, b * D : (b + 1) * D],
                            start=(t == 0),
                            stop=(t == NJP2 - 1),
                            perf_mode=DR,
                        )
            njp = pacc.tile([128, 512], F32, tag="njp")
            for s in range(nsw):
                nc.tensor.matmul(
                    njp[:, s : s + 1],
                    m01acc[:, s * 128 : (s + 1) * 128],
                    ones_col[:],
                    start=True,
                    stop=True,
                )
            # epilogue
            for s in range(nsw):
                nj = njp[:, s : s + 1]
                z = ep.tile([128, 1], F32, tag="z")
                nc.vector.tensor_scalar(z[:], nj, 0.0, None, op0=OP.is_gt)
                mx = ep.tile([128, 1], F32, tag="mx")
                nc.vector.tensor_scalar(mx[:], nj, 1.0, None, op0=OP.max)
                r = ep.tile([128, 1], F32, tag="r")
                nc.vector.reciprocal(r[:], mx[:])
                sc0 = ep.tile([128, 1], F32, tag="sc0")
                nc.vector.tensor_scalar(sc0[:], r[:], 0.5 * ASCALE, None, op0=OP.mult)
                sc = ep.tile([128, 1], F32, tag="sc")
                nc.vector.tensor_mul(sc[:], sc0[:], z[:])
                tt = ep.tile([128, 1], F32, tag="tt")
                nc.vector.tensor_scalar(tt[:], z[:], -0.5, 1.0, op0=OP.mult, op1=OP.add)
                for b in range(B):
                    xis = ep.tile([128, D], BF, tag="xis")
                    nc.sync.dma_start(out=xis[:], in_=xi[ibs + s, b])
                    ag = ep.tile([128, D], F32, tag="ag")
                    nc.scalar.activation(
                        ag[:], aggs[s * B + b][:], AF.Copy, bias=0.0, scale=sc[:]
                    )
                    yt = ep.tile([128, D], BF, tag="yt")
                    nc.vector.scalar_tensor_tensor(
                        yt[:], xis[:], tt[:], ag[:], op0=OP.mult, op1=OP.add
                    )
                    nc.sync.dma_start(out=y[ibs + s, b], in_=yt[:])
            ibs += nsw
    return nc


_NOSPLIT = ("InstEventSemaphore", "InstAllEngineBarrier")


def _split_waits(nc):
    """This walrus rejects >1 sync wait on TPB compute instructions; hoist
    extra waits onto per-wait EventSemaphore instructions just before."""
    nev = 0
    for f in nc.m.functions:
        for bb in f.blocks:
            out = []
            changed = False
            for inst in bb.instructions:
                si = getattr(inst, "sync_info", None)
                ow = list(si.on_wait) if si and si.on_wait else []
                if len(ow) >= 2 and type(inst).__name__ not in _NOSPLIT:
                    for w in ow[:-1]:
                        nev += 1
                        out.append(
                            mybir.InstEventSemaphore(
                                name=f"EVW-{nev}",
                                engine=inst.engine,
                                ins=[],
                                outs=[],
                                sync_info=mybir.SyncInfo(on_wait=[w], on_update=[]),
                            )
                        )
                    inst.sync_info = mybir.SyncInfo(
                        on_wait=ow[-1:], on_update=list(si.on_update or [])
                    )
                    changed = True
                out.append(inst)
            if changed:
                bb.instructions = out


def _host_prep(x, mask, emb_i, emb_j):
    m = mask.astype(bool)
    idx = np.where(m)[0]
    N = len(idx)
    assert N > 0
    chunk, nsub, per, windows, NJB, NJBp, NJP2 = _plan(N)
    BD = B * D

    def nrm(e):
        n = np.maximum(np.linalg.norm(e, axis=-1, keepdims=True), EPS)
        return (e / n * SCALE).astype(np.float32)

    ni8 = nrm(emb_i).astype(E4).astype(np.float32)  # keep f32 copy for emul
    nj8 = nrm(emb_j).astype(E4)

    NJ = NJBp * 128
    # j operands (global, block-pair DoubleRow layout)
    njp_rows = np.zeros((NJ, E), E4)
    njp_rows[:N] = nj8[idx]
    # [t, q, jj, k, p] -> [t, p, q, k, jj] -> [NJP2, 128, 4, 128]
    tmp = njp_rows.reshape(NJP2, 2, 128, 2, 128)  # [t, q, jj, k, p]
    nrmj_h = np.ascontiguousarray(tmp.transpose(0, 4, 1, 3, 2)).reshape(
        NJP2, 128, 4, 128
    )
    xsel = np.zeros((NJ, BD), np.float32)
    xsel[:N] = np.transpose(x[:, idx], (1, 0, 2)).reshape(N, BD)
    x8 = xsel.astype(E4)
    # [t, k, p, bd] -> [t, p, k, bd]
    xj_h = np.ascontiguousarray(
        x8.reshape(NJP2, 2, 128, BD).transpose(0, 2, 1, 3)
    )
    pj = np.full(NJ, -(10**6), np.int64)
    pj[:N] = idx

    slots = _slots(windows)
    in_maps = []
    meta = []
    for c in range(NCORES):
        s_c = min(c * chunk, N - 1)
        rows = np.clip(s_c + np.arange(per), 0, N - 1)
        gi = idx[rows]
        # nrmi [p, k, i]
        nis = ni8[gi].astype(E4)  # (per, E)
        nrmi_h = np.ascontiguousarray(nis.reshape(per, 2, 128).transpose(2, 1, 0))
        xi_h = np.ascontiguousarray(
            np.transpose(x[:, gi].reshape(B, nsub, 128, D), (1, 0, 2, 3))
        ).astype(BF16)
        # rotation (even block count so DR pairs stay aligned)
        r_c = 2 * (s_c // 256)
        pperm = (r_c // 2 + np.arange(NJP2)) % NJP2
        strips_h = np.ones((len(slots), 128, 256), BF16)
        ibs = 0
        for iw, W in enumerate(windows):
            pi = pj[:N][rows[ibs * 128 : ibs * 128 + W]]  # orig positions (real rows)
            pi = idx[rows[ibs * 128 : ibs * 128 + W]]
            for jw, lb, k in slots:
                if jw != iw:
                    continue
                g = (r_c + lb) % NJBp
                pjj = pj[g * 128 : (g + 1) * 128]
                dmat = np.abs(pjj[:, None] - pi[None, :])
                strips_h[k, :, :W] = (dmat > CHUNK).astype(BF16)
            ibs += W // 128
        in_maps.append(
            {
                "nrmj": nrmj_h[pperm],
                "nrmi": nrmi_h,
                "xj": xj_h[pperm],
                "xi": xi_h,
                "strips": strips_h,
            }
        )
        meta.append((s_c, min(N - s_c, chunk)))
    return in_maps, idx, N, meta


def kernel(x, mask, emb_i, emb_j):
    x = np.asarray(x, np.float32)
    mask = np.asarray(mask)
    emb_i = np.asarray(emb_i, np.float32)
    emb_j = np.asarray(emb_j, np.float32)

    in_maps, idx, N, meta = _host_prep(x, mask, emb_i, emb_j)
    chunk, nsub, per, windows, NJB, NJBp, NJP2 = _plan(N)
    nc = bass.Bass()
    _build(nc, N)
    _split_waits(nc)
    res = run_bass_kernel_spmd(nc, in_maps, list(range(NCORES)), trace=TRACE)
    LAST["res"] = res
    out = x.copy()
    for c in range(NCORES):
        s_c, cnt = meta[c]
        yc = res.results[c]["y"].astype(np.float32)  # [nsub, B, 128, D]
        yr = np.transpose(yc, (1, 0, 2, 3)).reshape(B, per, D)
        out[:, idx[s_c : s_c + cnt]] = yr[:, :cnt]
    return out


# revision 9
# speedup vs baseline: 1.5222x; 1.0886x over previous
"""Trainium2 Bass kernel for nn_LongRangeModule (gnn_message_passing).

Strategy (sequence-parallel over i, mask-compacted, fp8 DoubleRow):
  - Host: select masked-in rows (compaction), normalize embeddings and scale
    by 8 -> fp8 e4m3 (cos8 = 64*cos), pack j-operands in DoubleRow pair
    layout [pair, 128, k2, *], shard i-rows over 8 cores (640 rows each,
    5x128 subtiles in windows [256, 256, 128]).
  - j-blocks are rotated per core (by an even block count) so the near-band
    blocks (|pos_i - pos_j| <= 128 possible) sit at fixed LOCAL indices:
    6 slots per window get a far-mask strip; all other blocks are provably
    all-far and skip the strip entirely.
  - Device per core, per window, per j-block-pair t:
      cos8[j,i] = DoubleRow MM(nrmj8[t,q], nrmi8[:, :, win])      (PE, fp8)
      absc = |cos8| -> bf16                                        (ACT)
      src  = absc * strip   (near slots only)                      (DVE)
      wt8[:, q, :] = (src > 6.4) * src -> fp8                      (DVE, fused)
      m01pm = sign(src - 6.4) -> bf16 in {-1, +1}                  (ACT)
      agg[s,b] += DoubleRow MM(wt8[:, :, s], xj8[t][:, :, b])      (PE, fp8)
      njpm[:, s] += MM(m01pm[:, s], ones)   (N=1 matmuls, accumulated
        with start=False into a DVE-zeroed PSUM bank so the per-s groups
        can interleave without clearing each other's has_written bits)
    Window end: num_j = (njpm + NJ)/2  (exact; pad rows count as -1)
    Epilogue: y = t*x + sc*agg, t = 1-0.5*z, sc = (0.5/64)*z/max(nj,1),
    z = nj>0; y written bf16, upcast+scattered on host.
"""

import math
import sys

import numpy as np

try:
    import concourse.bass as bass
except ImportError:  # harness env may not have the repo on sys.path
    sys.path.insert(0, "/opt/trn_rl_repo")
    import concourse.bass as bass

import ml_dtypes
import concourse.mybir as mybir
from concourse.bass_utils import run_bass_kernel_spmd
from concourse.tile import TileContext

BF16 = ml_dtypes.bfloat16
E4 = ml_dtypes.float8_e4m3
F32 = mybir.dt.float32
BF = mybir.dt.bfloat16
F8 = mybir.dt.float8e4
AF = mybir.ActivationFunctionType
OP = mybir.AluOpType
DR = mybir.MatmulPerfMode.DoubleRow

B, L, D, E = 2, 8192, 512, 256
CHUNK, CUT, EPS = 128, 0.1, 1e-8
NCORES = 8
SCALE = 8.0  # nrm scale; cos8 = SCALE^2 * cos
CUT8 = CUT * SCALE * SCALE
ASCALE = 1.0 / (SCALE * SCALE)
NSLOT = 6  # near-band strip slots per window

TRACE = False
LAST = {}  # stash for test.py (exec_time_ns etc.)


def _plan(N):
    chunk = -(-N // NCORES)  # i-rows each core is responsible for
    nsub = -(-chunk // 128)  # 128-row subtiles per core
    per = nsub * 128
    windows = [256] * (nsub // 2) + ([128] if nsub % 2 else [])
    NJB = -(-N // 128)
    NJBp = NJB + (NJB & 1)
    NJP2 = NJBp // 2
    return chunk, nsub, per, windows, NJB, NJBp, NJP2


def _slots(windows):
    """[(iw, local_block, slot_index)] for near-band strips."""
    out = []
    k = 0
    ibs = 0
    for iw, w in enumerate(windows):
        for j in range(NSLOT):
            out.append((iw, ibs - 1 + j, k))
            k += 1
        ibs += w // 128
    return out


def _build(nc: bass.Bass, N: int):
    chunk, nsub, per, windows, NJB, NJBp, NJP2 = _plan(N)
    BD = B * D
    slotmap = {(iw, lb): k for iw, lb, k in _slots(windows)}

    nrmj = nc.dram_tensor("nrmj", [NJP2, 128, 4, 128], F8, kind="ExternalInput")
    nrmi = nc.dram_tensor("nrmi", [128, 2, per], F8, kind="ExternalInput")
    xj = nc.dram_tensor("xj", [NJP2, 128, 2, BD], F8, kind="ExternalInput")
    xi = nc.dram_tensor("xi", [nsub, B, 128, D], BF, kind="ExternalInput")
    strips = nc.dram_tensor(
        "strips", [NSLOT * len(windows), 128, 256], BF, kind="ExternalInput"
    )
    y = nc.dram_tensor("y", [nsub, B, 128, D], BF, kind="ExternalOutput")

    with (
        TileContext(nc) as tc,
        tc.tile_pool(name="res", bufs=1) as res,
        tc.tile_pool(name="stp", bufs=3) as stp,
        tc.tile_pool(name="wk", bufs=4) as wk,
        tc.tile_pool(name="wt", bufs=3) as wtp,
        tc.tile_pool(name="epi", bufs=3) as ep,
        tc.tile_pool(name="pcos", bufs=2, space="PSUM") as pcos,
        tc.tile_pool(name="pacc", bufs=1, space="PSUM") as pacc,
    ):
        # resident operands (small first so compute can start early)
        nrmi_sb = res.tile([128, 2, per], F8, tag="nrmi_sb")
        nc.sync.dma_start(out=nrmi_sb[:], in_=nrmi[:])
        ones_col = res.tile([128, 1], BF, tag="ones_col")
        nc.vector.memset(ones_col[:], 1.0)
        ncut = res.tile([128, 1], F32, tag="ncut")
        nc.vector.memset(ncut[:], -CUT8)
        nrmj_sb = []
        xj_sb = []
        for t in range(NJP2):
            nj = res.tile([128, 4, 128], F8, tag=f"nrmj{t}")
            nc.sync.dma_start(out=nj[:], in_=nrmj[t])
            nrmj_sb.append(nj)
            xt = res.tile([128, 2, BD], F8, tag=f"xj{t}")
            nc.sync.dma_start(out=xt[:], in_=xj[t])
            xj_sb.append(xt)

        NJtot = float(NJBp * 128)
        ibs = 0  # window's first subtile index
        for iw, W in enumerate(windows):
            nsw = W // 128
            lo = ibs * 128
            aggs = [
                pacc.tile([128, D], F32, tag=f"agg{k}", name=f"agg{k}")
                for k in range(nsw * B)
            ]
            njp = pacc.tile([128, 512], F32, tag="njp")
            nc.vector.memset(njp[:, :nsw], 0.0)
            for t in range(NJP2):
                wt8 = wtp.tile([128, 2, W], F8, tag="wt8")
                m01s = []
                for q in (0, 1):
                    lb = 2 * t + q
                    cos = pcos.tile([128, 512], F32, tag="cos")
                    nc.tensor.matmul(
                        cos[:, :W],
                        nrmj_sb[t][:, 2 * q : 2 * q + 2, :],
                        nrmi_sb[:, :, lo : lo + W],
                        start=True,
                        stop=True,
                        perf_mode=DR,
                    )
                    absc = wk.tile([128, W], BF, tag="absc")
                    nc.scalar.activation(absc[:], cos[:, :W], AF.Abs)
                    src = absc
                    k = slotmap.get((iw, lb))
                    if k is None:  # wrap: local block NJBp-1 is slot -1 of iw 0
                        k = slotmap.get((iw, lb - NJBp))
                    if k is not None:
                        strip = stp.tile([128, W], BF, tag="strip")
                        nc.sync.dma_start(out=strip[:], in_=strips[k][:, :W])
                        am = wk.tile([128, W], BF, tag="am")
                        nc.vector.tensor_mul(am[:], absc[:], strip[:])
                        src = am
                    nc.vector.scalar_tensor_tensor(
                        wt8[:, q, :], src[:], CUT8, src[:], op0=OP.is_gt, op1=OP.mult
                    )
                    m01pm = wk.tile([128, W], BF, tag="m01pm")
                    nc.scalar.activation(m01pm[:], src[:], AF.Sign, bias=ncut[:])
                    m01s.append(m01pm)
                for s in range(nsw):
                    for b in range(B):
                        nc.tensor.matmul(
                            aggs[s * B + b][:],
                            wt8[:, :, s * 128 : (s + 1) * 128],
                            xj_sb[t][:, :, b * D : (b + 1) * D],
                            start=(t == 0),
                            stop=(t == NJP2 - 1),
                            perf_mode=DR,
                        )
                for q in (0, 1):
                    for s in range(nsw):
                        nc.tensor.matmul(
                            njp[:, s : s + 1],
                            m01s[q][:, s * 128 : (s + 1) * 128],
                            ones_col[:],
                            start=False,
                            stop=(t == NJP2 - 1 and q == 1),
                            skip_group_check=True,
                        )
            # epilogue
            for s in range(nsw):
                nj = ep.tile([128, 1], F32, tag="nj")
                nc.vector.tensor_scalar(
                    nj[:], njp[:, s : s + 1], NJtot, 0.5, op0=OP.add, op1=OP.mult
                )
                z = ep.tile([128, 1], F32, tag="z")
                nc.vector.tensor_scalar(z[:], nj[:], 0.0, None, op0=OP.is_gt)
                mx = ep.tile([128, 1], F32, tag="mx")
                nc.vector.tensor_scalar(mx[:], nj[:], 1.0, None, op0=OP.max)
                r = ep.tile([128, 1], F32, tag="r")
                nc.vector.reciprocal(r[:], mx[:])
                sc0 = ep.tile([128, 1], F32, tag="sc0")
                nc.vector.tensor_scalar(sc0[:], r[:], 0.5 * ASCALE, None, op0=OP.mult)
                sc = ep.tile([128, 1], F32, tag="sc")
                nc.vector.tensor_mul(sc[:], sc0[:], z[:])
                tt = ep.tile([128, 1], F32, tag="tt")
                nc.vector.tensor_scalar(tt[:], z[:], -0.5, 1.0, op0=OP.mult, op1=OP.add)
                for b in range(B):
                    xis = ep.tile([128, D], BF, tag="xis")
                    nc.sync.dma_start(out=xis[:], in_=xi[ibs + s, b])
                    ag = ep.tile([128, D], F32, tag="ag")
                    nc.scalar.activation(
                        ag[:], aggs[s * B + b][:], AF.Copy, bias=0.0, scale=sc[:]
                    )
                    yt = ep.tile([128, D], BF, tag="yt")
                    nc.vector.scalar_tensor_tensor(
                        yt[:], xis[:], tt[:], ag[:], op0=OP.mult, op1=OP.add
                    )
                    nc.sync.dma_start(out=y[ibs + s, b], in_=yt[:])
            ibs += nsw
    return nc


_NOSPLIT = ("InstEventSemaphore", "InstAllEngineBarrier")


def _split_waits(nc):
    """This walrus rejects >1 sync wait on TPB compute instructions; hoist
    extra waits onto per-wait EventSemaphore instructions just before."""
    nev = 0
    for f in nc.m.functions:
        for bb in f.blocks:
            out = []
            changed = False
            for inst in bb.instructions:
                si = getattr(inst, "sync_info", None)
                ow = list(si.on_wait) if si and si.on_wait else []
                if len(ow) >= 2 and type(inst).__name__ not in _NOSPLIT:
                    for w in ow[:-1]:
                        nev += 1
                        out.append(
                            mybir.InstEventSemaphore(
                                name=f"EVW-{nev}",
                                engine=inst.engine,
                                ins=[],
                                outs=[],
                                sync_info=mybir.SyncInfo(on_wait=[w], on_update=[]),
                            )
                        )
                    inst.sync_info = mybir.SyncInfo(
                        on_wait=ow[-1:], on_update=list(si.on_update or [])
                    )
                    changed = True
                out.append(inst)
            if changed:
                bb.instructions = out


def _host_prep(x, mask, emb_i, emb_j):
    m = mask.astype(bool)
    idx = np.where(m)[0]
    N = len(idx)
    assert N > 0
    chunk, nsub, per, windows, NJB, NJBp, NJP2 = _plan(N)
    BD = B * D

    def nrm(e):
        n = np.maximum(np.linalg.norm(e, axis=-1, keepdims=True), EPS)
        return (e / n * SCALE).astype(np.float32)

    ni8 = nrm(emb_i).astype(E4).astype(np.float32)  # keep f32 copy for emul
    nj8 = nrm(emb_j).astype(E4)

    NJ = NJBp * 128
    # j operands (global, block-pair DoubleRow layout)
    njp_rows = np.zeros((NJ, E), E4)
    njp_rows[:N] = nj8[idx]
    # [t, q, jj, k, p] -> [t, p, q, k, jj] -> [NJP2, 128, 4, 128]
    tmp = njp_rows.reshape(NJP2, 2, 128, 2, 128)  # [t, q, jj, k, p]
    nrmj_h = np.ascontiguousarray(tmp.transpose(0, 4, 1, 3, 2)).reshape(
        NJP2, 128, 4, 128
    )
    xsel = np.zeros((NJ, BD), np.float32)
    xsel[:N] = np.transpose(x[:, idx], (1, 0, 2)).reshape(N, BD)
    x8 = xsel.astype(E4)
    # [t, k, p, bd] -> [t, p, k, bd]
    xj_h = np.ascontiguousarray(
        x8.reshape(NJP2, 2, 128, BD).transpose(0, 2, 1, 3)
    )
    pj = np.full(NJ, -(10**6), np.int64)
    pj[:N] = idx

    slots = _slots(windows)
    in_maps = []
    meta = []
    for c in range(NCORES):
        s_c = min(c * chunk, N - 1)
        rows = np.clip(s_c + np.arange(per), 0, N - 1)
        gi = idx[rows]
        # nrmi [p, k, i]
        nis = ni8[gi].astype(E4)  # (per, E)
        nrmi_h = np.ascontiguousarray(nis.reshape(per, 2, 128).transpose(2, 1, 0))
        xi_h = np.ascontiguousarray(
            np.transpose(x[:, gi].reshape(B, nsub, 128, D), (1, 0, 2, 3))
        ).astype(BF16)
        # rotation (even block count so DR pairs stay aligned)
        r_c = 2 * (s_c // 256)
        pperm = (r_c // 2 + np.arange(NJP2)) % NJP2
        strips_h = np.ones((len(slots), 128, 256), BF16)
        ibs = 0
        for iw, W in enumerate(windows):
            pi = pj[:N][rows[ibs * 128 : ibs * 128 + W]]  # orig positions (real rows)
            pi = idx[rows[ibs * 128 : ibs * 128 + W]]
            for jw, lb, k in slots:
                if jw != iw:
                    continue
                g = (r_c + lb) % NJBp
                pjj = pj[g * 128 : (g + 1) * 128]
                dmat = np.abs(pjj[:, None] - pi[None, :])
                strips_h[k, :, :W] = (dmat > CHUNK).astype(BF16)
            ibs += W // 128
        in_maps.append(
            {
                "nrmj": nrmj_h[pperm],
                "nrmi": nrmi_h,
                "xj": xj_h[pperm],
                "xi": xi_h,
                "strips": strips_h,
            }
        )
        meta.append((s_c, min(N - s_c, chunk)))
    return in_maps, idx, N, meta


def kernel(x, mask, emb_i, emb_j):
    x = np.asarray(x, np.float32)
    mask = np.asarray(mask)
    emb_i = np.asarray(emb_i, np.float32)
    emb_j = np.asarray(emb_j, np.float32)

    in_maps, idx, N, meta = _host_prep(x, mask, emb_i, emb_j)
    chunk, nsub, per, windows, NJB, NJBp, NJP2 = _plan(N)
    nc = bass.Bass()
    _build(nc, N)
    _split_waits(nc)
    res = run_bass_kernel_spmd(nc, in_maps, list(range(NCORES)), trace=TRACE)
    LAST["res"] = res
    out = x.copy()
    for c in range(NCORES):
        s_c, cnt = meta[c]
        yc = res.results[c]["y"].astype(np.float32)  # [nsub, B, 128, D]
        yr = np.transpose(yc, (1, 0, 2, 3)).reshape(B, per, D)
        out[:, idx[s_c : s_c + cnt]] = yr[:, :cnt]
    return out


# revision 12
# speedup vs baseline: 1.8190x; 1.1950x over previous
"""Trainium2 Bass kernel for nn_LongRangeModule (gnn_message_passing).

Strategy (sequence-parallel over i, mask-compacted, fp8 DoubleRow):
  - Host: select masked-in rows (compaction), normalize embeddings and scale
    by 8 -> fp8 e4m3 (cos8 = 64*cos), pack j-operands in DoubleRow pair
    layout [pair, 128, k2, *], shard i-rows over 8 cores (640 rows each,
    5x128 subtiles in windows [256, 256, 128]).
  - j-blocks are rotated per core (by an even block count) so the near-band
    blocks (|pos_i - pos_j| <= 128 possible) sit at fixed LOCAL indices:
    6 slots per window get a far-mask strip; all other blocks are provably
    all-far and skip the strip entirely.
  - Device per core, per window, per j-block-pair t:
      cos8[j,i] = DoubleRow MM(nrmj8[t,q], nrmi8[:, :, win])      (PE, fp8)
      absc = |cos8| -> bf16                                        (ACT)
      src  = absc * strip   (near slots only)                      (DVE)
      wt8[:, q, :] = (src > 6.4) * src -> fp8                      (DVE, fused)
      m01pm = sign(src - 6.4) -> bf16 in {-1, +1}                  (ACT)
      agg[s,b] += DoubleRow MM(wt8[:, :, s], xj8[t][:, :, b])      (PE, fp8)
      njpm[:, s] += MM(m01pm[:, s], ones)   (N=1 matmuls, accumulated
        with start=False into a DVE-zeroed PSUM bank so the per-s groups
        can interleave without clearing each other's has_written bits)
    Window end: num_j = (njpm + NJ)/2  (exact; pad rows count as -1)
    Epilogue: y = t*x + sc*agg, t = 1-0.5*z, sc = (0.5/64)*z/max(nj,1),
    z = nj>0; y written bf16, upcast+scattered on host.
"""

import math
import sys

import numpy as np

try:
    import concourse.bass as bass
except ImportError:  # harness env may not have the repo on sys.path
    sys.path.insert(0, "/opt/trn_rl_repo")
    import concourse.bass as bass

import ml_dtypes
import concourse.mybir as mybir
from concourse.bass_utils import run_bass_kernel_spmd
from concourse.tile import TileContext

BF16 = ml_dtypes.bfloat16
E4 = ml_dtypes.float8_e4m3
F32 = mybir.dt.float32
BF = mybir.dt.bfloat16
F8 = mybir.dt.float8e4
AF = mybir.ActivationFunctionType
OP = mybir.AluOpType
DR = mybir.MatmulPerfMode.DoubleRow

B, L, D, E = 2, 8192, 512, 256
CHUNK, CUT, EPS = 128, 0.1, 1e-8
NCORES = 8
SCALE = 8.0  # nrm scale; cos8 = SCALE^2 * cos
CUT8 = CUT * SCALE * SCALE
ASCALE = 1.0 / (SCALE * SCALE)
NSLOT = 6  # near-band strip slots per window

TRACE = False
LAST = {}  # stash for test.py (exec_time_ns etc.)


def _plan(N):
    chunk = -(-N // NCORES)  # i-rows each core is responsible for
    nsub = -(-chunk // 128)  # 128-row subtiles per core
    per = nsub * 128
    windows = [256] * (nsub // 2) + ([128] if nsub % 2 else [])
    NJB = -(-N // 128)
    NJBp = NJB + (NJB & 1)
    NJP2 = NJBp // 2
    return chunk, nsub, per, windows, NJB, NJBp, NJP2


def _slots(windows):
    """[(iw, local_block, slot_index)] for near-band strips."""
    out = []
    k = 0
    ibs = 0
    for iw, w in enumerate(windows):
        for j in range(NSLOT):
            out.append((iw, ibs - 1 + j, k))
            k += 1
        ibs += w // 128
    return out


def _build(nc: bass.Bass, N: int):
    chunk, nsub, per, windows, NJB, NJBp, NJP2 = _plan(N)
    BD = B * D
    slotmap = {(iw, lb): k for iw, lb, k in _slots(windows)}

    nrmj = nc.dram_tensor("nrmj", [NJP2, 128, 4, 128], F8, kind="ExternalInput")
    nrmi = nc.dram_tensor("nrmi", [128, 2, per], F8, kind="ExternalInput")
    xj = nc.dram_tensor("xj", [NJP2, 128, 2, BD], F8, kind="ExternalInput")
    xi = nc.dram_tensor("xi", [nsub, B, 128, D], BF, kind="ExternalInput")
    strips = nc.dram_tensor(
        "strips", [NSLOT * len(windows), 128, 256], BF, kind="ExternalInput"
    )
    y = nc.dram_tensor("y", [nsub, B, 128, D], BF, kind="ExternalOutput")

    with (
        TileContext(nc) as tc,
        tc.tile_pool(name="res", bufs=1) as res,
        tc.tile_pool(name="stp", bufs=3) as stp,
        tc.tile_pool(name="wk", bufs=4) as wk,
        tc.tile_pool(name="wt", bufs=3) as wtp,
        tc.tile_pool(name="epi", bufs=3) as ep,
        tc.tile_pool(name="pcos", bufs=2, space="PSUM") as pcos,
        tc.tile_pool(name="pacc", bufs=1, space="PSUM") as pacc,
    ):
        # resident operands (small first so compute can start early)
        nrmi_sb = res.tile([128, 2, per], F8, tag="nrmi_sb")
        nc.sync.dma_start(out=nrmi_sb[:], in_=nrmi[:])
        ones_col = res.tile([128, 1], BF, tag="ones_col")
        nc.vector.memset(ones_col[:], 1.0)
        ncut = res.tile([128, 1], F32, tag="ncut")
        nc.vector.memset(ncut[:], -CUT8)
        # per-window pair order: strip-free (far) pairs first so the strip
        # DMAs are never on the critical path of the PSUM accumulation chain
        def near(iw, lb):
            k = slotmap.get((iw, lb))
            if k is None:  # wrap: local block NJBp-1 is slot -1 of iw 0
                k = slotmap.get((iw, lb - NJBp))
            return k

        orders = []
        for iw in range(len(windows)):
            ts = list(range(NJP2))
            ts.sort(key=lambda t: (near(iw, 2 * t) is not None)
                    or (near(iw, 2 * t + 1) is not None))
            orders.append(ts)

        nrmj_sb = [None] * NJP2
        xj_sb = [None] * NJP2
        for t in orders[0]:  # load in first-use order
            nj = res.tile([128, 4, 128], F8, tag=f"nrmj{t}", name=f"nrmj{t}")
            nc.sync.dma_start(out=nj[:], in_=nrmj[t])
            nrmj_sb[t] = nj
        for t in orders[0]:
            xt = res.tile([128, 2, BD], F8, tag=f"xj{t}", name=f"xj{t}")
            nc.sync.dma_start(out=xt[:], in_=xj[t])
            xj_sb[t] = xt

        NJtot = float(NJBp * 128)
        ibs = 0  # window's first subtile index
        for iw, W in enumerate(windows):
            nsw = W // 128
            lo = ibs * 128
            aggs = [
                pacc.tile([128, D], F32, tag=f"agg{k}", name=f"agg{k}")
                for k in range(nsw * B)
            ]
            njp = pacc.tile([128, 512], F32, tag="njp")
            nc.vector.memset(njp[:, :nsw], 0.0)
            order = orders[iw]
            for ti, t in enumerate(order):
                first, last = ti == 0, ti == NJP2 - 1
                wt8 = wtp.tile([128, 2, W], F8, tag="wt8")
                abst = wk.tile([128, 2, W], BF, tag="absc2", name="absc2")
                srcs = []
                anynear = False
                for q in (0, 1):
                    lb = 2 * t + q
                    cos = pcos.tile([128, 512], F32, tag="cos")
                    nc.tensor.matmul(
                        cos[:, :W],
                        nrmj_sb[t][:, 2 * q : 2 * q + 2, :],
                        nrmi_sb[:, :, lo : lo + W],
                        start=True,
                        stop=True,
                        perf_mode=DR,
                    )
                    nc.scalar.activation(abst[:, q, :], cos[:, :W], AF.Abs)
                    src = abst[:, q, :]
                    k = near(iw, lb)
                    if k is not None:
                        anynear = True
                        strip = stp.tile([128, W], BF, tag="strip")
                        nc.sync.dma_start(out=strip[:], in_=strips[k][:, :W])
                        am = wk.tile([128, W], BF, tag="am")
                        nc.vector.tensor_mul(am[:], abst[:, q, :], strip[:])
                        src = am[:]
                    nc.vector.scalar_tensor_tensor(
                        wt8[:, q, :], src, CUT8, src, op0=OP.is_gt, op1=OP.mult
                    )
                    srcs.append(src)
                # sign(src - CUT8) -> {-1, +1}; one batched op for far pairs
                m01pm = wk.tile([128, 2, W], BF, tag="m01pm", name="m01pm")
                if anynear:
                    for q in (0, 1):
                        nc.scalar.activation(
                            m01pm[:, q, :], srcs[q], AF.Sign, bias=ncut[:]
                        )
                else:
                    nc.scalar.activation(m01pm[:], abst[:], AF.Sign, bias=ncut[:])
                for s in range(nsw):
                    for b in range(B):
                        nc.tensor.matmul(
                            aggs[s * B + b][:],
                            wt8[:, :, s * 128 : (s + 1) * 128],
                            xj_sb[t][:, :, b * D : (b + 1) * D],
                            start=first,
                            stop=last,
                            perf_mode=DR,
                        )
                for q in (0, 1):
                    for s in range(nsw):
                        nc.tensor.matmul(
                            njp[:, s : s + 1],
                            m01pm[:, q, s * 128 : (s + 1) * 128],
                            ones_col[:],
                            start=False,
                            stop=(last and q == 1),
                            skip_group_check=True,
                        )
            # epilogue
            for s in range(nsw):
                nj = ep.tile([128, 1], F32, tag="nj")
                nc.vector.tensor_scalar(
                    nj[:], njp[:, s : s + 1], NJtot, 0.5, op0=OP.add, op1=OP.mult
                )
                z = ep.tile([128, 1], F32, tag="z")
                nc.vector.tensor_scalar(z[:], nj[:], 0.0, None, op0=OP.is_gt)
                mx = ep.tile([128, 1], F32, tag="mx")
                nc.vector.tensor_scalar(mx[:], nj[:], 1.0, None, op0=OP.max)
                r = ep.tile([128, 1], F32, tag="r")
                nc.vector.reciprocal(r[:], mx[:])
                sc0 = ep.tile([128, 1], F32, tag="sc0")
                nc.vector.tensor_scalar(sc0[:], r[:], 0.5 * ASCALE, None, op0=OP.mult)
                sc = ep.tile([128, 1], F32, tag="sc")
                nc.vector.tensor_mul(sc[:], sc0[:], z[:])
                tt = ep.tile([128, 1], F32, tag="tt")
                nc.vector.tensor_scalar(tt[:], z[:], -0.5, 1.0, op0=OP.mult, op1=OP.add)
                for b in range(B):
                    xis = ep.tile([128, D], BF, tag="xis")
                    nc.sync.dma_start(out=xis[:], in_=xi[ibs + s, b])
                    ag = ep.tile([128, D], F32, tag="ag")
                    nc.scalar.activation(
                        ag[:], aggs[s * B + b][:], AF.Copy, bias=0.0, scale=sc[:]
                    )
                    yt = ep.tile([128, D], BF, tag="yt")
                    nc.vector.scalar_tensor_tensor(
                        yt[:], xis[:], tt[:], ag[:], op0=OP.mult, op1=OP.add
                    )
                    nc.sync.dma_start(out=y[ibs + s, b], in_=yt[:])
            ibs += nsw
    return nc


_NOSPLIT = ("InstEventSemaphore", "InstAllEngineBarrier")


def _split_waits(nc):
    """This walrus rejects >1 sync wait on TPB compute instructions; hoist
    extra waits onto per-wait EventSemaphore instructions just before."""
    nev = 0
    for f in nc.m.functions:
        for bb in f.blocks:
            out = []
            changed = False
            for inst in bb.instructions:
                si = getattr(inst, "sync_info", None)
                ow = list(si.on_wait) if si and si.on_wait else []
                if len(ow) >= 2 and type(inst).__name__ not in _NOSPLIT:
                    for w in ow[:-1]:
                        nev += 1
                        out.append(
                            mybir.InstEventSemaphore(
                                name=f"EVW-{nev}",
                                engine=inst.engine,
                                ins=[],
                                outs=[],
                                sync_info=mybir.SyncInfo(on_wait=[w], on_update=[]),
                            )
                        )
                    inst.sync_info = mybir.SyncInfo(
                        on_wait=ow[-1:], on_update=list(si.on_update or [])
                    )
                    changed = True
                out.append(inst)
            if changed:
                bb.instructions = out


def _host_prep(x, mask, emb_i, emb_j):
    m = mask.astype(bool)
    idx = np.where(m)[0]
    N = len(idx)
    assert N > 0
    chunk, nsub, per, windows, NJB, NJBp, NJP2 = _plan(N)
    BD = B * D

    def nrm(e):
        n = np.maximum(np.linalg.norm(e, axis=-1, keepdims=True), EPS)
        return (e / n * SCALE).astype(np.float32)

    ni8 = nrm(emb_i).astype(E4).astype(np.float32)  # keep f32 copy for emul
    nj8 = nrm(emb_j).astype(E4)

    NJ = NJBp * 128
    # j operands (global, block-pair DoubleRow layout)
    njp_rows = np.zeros((NJ, E), E4)
    njp_rows[:N] = nj8[idx]
    # [t, q, jj, k, p] -> [t, p, q, k, jj] -> [NJP2, 128, 4, 128]
    tmp = njp_rows.reshape(NJP2, 2, 128, 2, 128)  # [t, q, jj, k, p]
    nrmj_h = np.ascontiguousarray(tmp.transpose(0, 4, 1, 3, 2)).reshape(
        NJP2, 128, 4, 128
    )
    xsel = np.zeros((NJ, BD), np.float32)
    xsel[:N] = np.transpose(x[:, idx], (1, 0, 2)).reshape(N, BD)
    x8 = xsel.astype(E4)
    # [t, k, p, bd] -> [t, p, k, bd]
    xj_h = np.ascontiguousarray(
        x8.reshape(NJP2, 2, 128, BD).transpose(0, 2, 1, 3)
    )
    pj = np.full(NJ, -(10**6), np.int64)
    pj[:N] = idx

    slots = _slots(windows)
    in_maps = []
    meta = []
    for c in range(NCORES):
        s_c = min(c * chunk, N - 1)
        rows = np.clip(s_c + np.arange(per), 0, N - 1)
        gi = idx[rows]
        # nrmi [p, k, i]
        nis = ni8[gi].astype(E4)  # (per, E)
        nrmi_h = np.ascontiguousarray(nis.reshape(per, 2, 128).transpose(2, 1, 0))
        xi_h = np.ascontiguousarray(
            np.transpose(x[:, gi].reshape(B, nsub, 128, D), (1, 0, 2, 3))
        ).astype(BF16)
        # rotation (even block count so DR pairs stay aligned)
        r_c = 2 * (s_c // 256)
        pperm = (r_c // 2 + np.arange(NJP2)) % NJP2
        strips_h = np.ones((len(slots), 128, 256), BF16)
        ibs = 0
        for iw, W in enumerate(windows):
            pi = pj[:N][rows[ibs * 128 : ibs * 128 + W]]  # orig positions (real rows)
            pi = idx[rows[ibs * 128 : ibs * 128 + W]]
            for jw, lb, k in slots:
                if jw != iw:
                    continue
                g = (r_c + lb) % NJBp
                pjj = pj[g * 128 : (g + 1) * 128]
                dmat = np.abs(pjj[:, None] - pi[None, :])
                strips_h[k, :, :W] = (dmat > CHUNK).astype(BF16)
            ibs += W // 128
        in_maps.append(
            {
                "nrmj": nrmj_h[pperm],
                "nrmi": nrmi_h,
                "xj": xj_h[pperm],
                "xi": xi_h,
                "strips": strips_h,
            }
        )
        meta.append((s_c, min(N - s_c, chunk)))
    return in_maps, idx, N, meta


def kernel(x, mask, emb_i, emb_j):
    x = np.asarray(x, np.float32)
    mask = np.asarray(mask)
    emb_i = np.asarray(emb_i, np.float32)
    emb_j = np.asarray(emb_j, np.float32)

    in_maps, idx, N, meta = _host_prep(x, mask, emb_i, emb_j)
    chunk, nsub, per, windows, NJB, NJBp, NJP2 = _plan(N)
    nc = bass.Bass()
    _build(nc, N)
    _split_waits(nc)
    res = run_bass_kernel_spmd(nc, in_maps, list(range(NCORES)), trace=TRACE)
    LAST["res"] = res
    out = x.copy()
    for c in range(NCORES):
        s_c, cnt = meta[c]
        yc = res.results[c]["y"].astype(np.float32)  # [nsub, B, 128, D]
        yr = np.transpose(yc, (1, 0, 2, 3)).reshape(B, per, D)
        out[:, idx[s_c : s_c + cnt]] = yr[:, :cnt]
    return out
